# revision 18
# baseline (speedup 1.0000x reference)
"""Trainium2 Bass kernel for the NeuralODE problem (v2).

Reference computation (per batch row y of dim D=64):
    f(y) = tanh(y @ W1) @ W2            (H=256 hidden; b1=b2=0 in this problem)
    49 intervals x 8 RK4 substeps with h = dt/8; save state each interval
    out[t] = sol[t] @ Wfc               (O=32, bfc=0)

Strategy (pure data parallel over 8 cores, B=16384 -> 2048/core):
  - State kept on-chip in packed transposed layout, one tile per stream
    (2 streams x 1024 batch rows):
      ys[s] [128, 512] f32: partitions 0:64 / 64:128 = y[d, b] for the
      stream's two 512-row batch halves; yr[s] is a float32r shadow that
      the layer-1 matmuls stream from at 1 cycle/row.
  - ScalarE tanh is the bottleneck (4 evals x 256 H x 2048 rows / substep
    = 2.1M elements at 128/cycle @1.2GHz).  All other engines are kept
    under it:
      * G PSUM tile is [128, 2048] (4 banks, both H-blocks m in the free
        dim) so each f-eval is ONE activation instruction (N=2048),
        amortizing the ~350-cycle ScalarE instruction overhead.
      * layer-1 y-term: f32r matmuls (1 cyc/row), K=64 row-tile pairs.
      * layer-1 k-term, layer-2: fp16 (1 cyc/row, 4x the mantissa of
        bf16), M=64 column-tile pairs.
      * K_i accumulates into bank 0 of the same g tile after the
        activation read (PSUM is fully used by the two streams' g tiles).
  - RK4 algebra (no y+c*k intermediate is ever formed):
      G_i = W1^T y + W1^T (c_i K_{i-1})   (PSUM-accumulated matmul pair)
      H_i = tanh(G_i)                     (ScalarE, one instr per eval)
      K_i = W2^T H_i                      (fp16 kb copy feeds the chain)
      y += h/6 K1 + h/3 K2 + h/3 K3 + h/6 K4
        as four DVE scalar_tensor_tensor ops reading each K_i straight
        from PSUM at full fp32 precision (no 16-matmul update group).
  - Projection out[t] = Wfc^T y runs as 4 column-tiled matmuls per
    interval (one per (stream, batch-half)), staged and DMA'd per
    interval.
"""

from contextlib import ExitStack

import numpy as np

B_FULL = 16384
N_CORES = 8
B_CORE = B_FULL // N_CORES          # 2048
HALF = B_CORE // 2                  # 1024 batch rows per partition-half
D = 64
H = 256
O = 32
T_FULL = 50
N_SUB = 8
N_STREAMS = 2
SFREE = HALF // N_STREAMS           # 512 free columns per stream tile
GW = 4 * SFREE                      # 2048: merged G tile width


def _split_multiwait_instructions(nc):
    """The walrus build in this container supports at most ONE semaphore
    wait per hardware instruction ("Too many sync wait commands").  Tile's
    sem-assignment can attach several.  Splitting is sound: insert NOPs on
    the same engine immediately before the instruction, each carrying one
    of the extra waits — the engine stalls through them sequentially at
    exactly the point it would have stalled anyway.
    """
    import bass_rust
    from concourse import mybir

    n = 0
    for fn in nc.m.functions:
        for bb in fn.blocks:
            out = []
            for inst in bb.instructions:
                si = inst.sync_info
                waits = list(si.on_wait) if si is not None and si.on_wait else []
                if len(waits) > 1:
                    for w in waits[:-1]:
                        n += 1
                        nop = bass_rust.InstNoOp(
                            name=f"{inst.name}-ws{n}", ins=[], outs=[])
                        nop.engine = inst.engine
                        nop.sync_info = mybir.SyncInfo(on_wait=[w], on_update=[])
                        nc.inst_map[nop.name] = nop
                        out.append(nop)
                    inst.sync_info = mybir.SyncInfo(
                        on_wait=[waits[-1]],
                        on_update=list(si.on_update) if si.on_update else [])
                out.append(inst)
            bb.instructions = out
    return n


def _build_kernel(n_intervals):
    import concourse.bass as bass
    import concourse.tile as tile
    from concourse import mybir

    f32 = mybir.dt.float32
    f32r = mybir.dt.float32r
    fp16 = mybir.dt.float16
    AF = mybir.ActivationFunctionType
    ALU = mybir.AluOpType
    ET = mybir.EngineType

    T = T_FULL
    nc = bass.Bass(trn_type="TRN2")

    RBLOB = H                           # w1f (f32r: 1 cyc/row layer-1 y-term)
    FBLOB = O + 2 + HALF                # wfcs|h6|h3|y0p
    BBLOB = 2 * H + 2 * D               # w1h2|w1h1|w2k
    rblob_d = nc.dram_tensor("rblob", [128, RBLOB], f32r, kind="ExternalInput")
    fblob_d = nc.dram_tensor("fblob", [128, FBLOB], f32, kind="ExternalInput")
    bblob_d = nc.dram_tensor("bblob", [128, BBLOB], fp16, kind="ExternalInput")
    out_d = nc.dram_tensor("out", [T, 128, SFREE], f32, kind="ExternalOutput")

    with tile.TileContext(nc) as tc, ExitStack() as ctx:
        persist = ctx.enter_context(tc.tile_pool(name="persist", bufs=1))
        hpool = ctx.enter_context(tc.tile_pool(name="hpool", bufs=6))
        kbpool = ctx.enter_context(tc.tile_pool(name="kbpool", bufs=4))
        accpool = ctx.enter_context(tc.tile_pool(name="accpool", bufs=2))
        stpool = ctx.enter_context(tc.tile_pool(name="stpool", bufs=2))
        gpsum = ctx.enter_context(tc.tile_pool(name="gpsum", bufs=2, space="PSUM"))

        rblob = persist.tile([128, RBLOB], f32r, tag="rblob", name="rblob")
        fblob = persist.tile([128, FBLOB], f32, tag="fblob", name="fblob")
        bblob = persist.tile([128, BBLOB], fp16, tag="bblob", name="bblob")
        nc.sync.dma_start(out=rblob, in_=rblob_d[:])
        nc.sync.dma_start(out=fblob, in_=fblob_d[:])
        nc.sync.dma_start(out=bblob, in_=bblob_d[:])

        def fcut(n):
            fcut.o += n
            return fblob[:, fcut.o - n:fcut.o]
        fcut.o = 0

        def bcut(n):
            bcut.o += n
            return bblob[:, bcut.o - n:bcut.o]
        bcut.o = 0

        w1f = rblob[:, 0:H]
        wfcs = fcut(O)
        h6col = fcut(1)
        h3col = fcut(1)
        y0sb = fcut(HALF)
        w1h2 = bcut(H)
        w1h1 = bcut(H)
        w2k = bcut(2 * D).rearrange("p (k d) -> p k d", k=2)

        # state lives in its own tiles (updated in place each substep);
        # yr is the f32r shadow the layer-1 matmuls stream from
        ys = [persist.tile([128, SFREE], f32, tag=f"ystate{s}", name=f"ystate{s}")
              for s in range(N_STREAMS)]
        yr = [persist.tile([128, SFREE], f32r, tag=f"yr{s}", name=f"yr{s}")
              for s in range(N_STREAMS)]
        for s in range(N_STREAMS):
            nc.vector.tensor_copy(ys[s], y0sb[:, s * SFREE:(s + 1) * SFREE])
            nc.vector.tensor_copy(yr[s], y0sb[:, s * SFREE:(s + 1) * SFREE])

        def project_and_store(dest_ap):
            """out[t, 32j+o, c] = sum_d Wfc[d, o] y[d, b],  j = 2s + hh,
            b = 1024*hh + 512*s + c.  One column-tiled matmul per j, each
            in its own PSUM bank (two fresh g-pool tiles; two allocations
            keep the stream<->slot parity of the g pool)."""
            stage = stpool.tile([128, SFREE], f32, tag="stage", name="stage")
            for s in range(N_STREAMS):
                pt = gpsum.tile([128, GW], f32, tag="g", name="projp")
                for hh in range(2):
                    j = 2 * s + hh
                    hsl = slice(64 * hh, 64 * (hh + 1))
                    psl = slice(32 * j, 32 * (j + 1))
                    osl = slice(2 * SFREE + hh * SFREE, 2 * SFREE + (hh + 1) * SFREE)
                    nc.tensor.matmul(pt[psl, osl], wfcs[hsl, :], ys[s][hsl, :],
                                     start=True, stop=True,
                                     tile_position=(64 * hh, 32 * j))
                    nc.vector.tensor_copy(stage[psl, :], pt[psl, osl])
            nc.sync.dma_start(out=dest_ap[0], in_=stage)

        def substep(h6, h3):
            """One RK4 substep for both streams, emission interleaved.

            The RK4 combination runs on DVE, reading each K_i straight
            from PSUM:  acc = K1*h/6 + y;  acc += K2*h/3; acc += K3*h/3;
            y = K4*h/6 + acc.
            """
            kb_prev = [None] * N_STREAMS
            acc = [None] * N_STREAMS
            for i in range(4):
                w1c = None if i == 0 else (w1h2 if i < 3 else w1h1)
                ci = h6[:, 0:1] if i in (0, 3) else h3[:, 0:1]
                for s in range(N_STREAMS):
                    g = gpsum.tile([128, GW], f32, tag="g", name="g")
                    # y-terms first: they depend only on yr, so the PE can
                    # run them while DVE still copies kb for the k-terms.
                    for m in range(2):
                        for hh in range(2):
                            hsl = slice(64 * hh, 64 * (hh + 1))
                            osl = slice(1024 * m + 512 * hh,
                                        1024 * m + 512 * (hh + 1))
                            nc.tensor.matmul(
                                g[:, osl],
                                w1f[hsl, 128 * m:128 * (m + 1)],
                                yr[s][hsl, :],
                                start=True, stop=(w1c is None),
                            )
                    if w1c is not None:
                        for m in range(2):
                            for hh in range(2):
                                hsl = slice(64 * hh, 64 * (hh + 1))
                                osl = slice(1024 * m + 512 * hh,
                                            1024 * m + 512 * (hh + 1))
                                nc.tensor.matmul(
                                    g[:, osl],
                                    w1c[hsl, 128 * m:128 * (m + 1)],
                                    kb_prev[s][hsl, :],
                                    start=False, stop=True,
                                )
                    ht = hpool.tile([128, GW], fp16, tag="h", name="h")
                    nc.scalar.activation(ht, g, AF.Tanh)
                    # K_i -> bank 0 of this eval's g tile (free after the
                    # activation read)
                    for hh in range(2):
                        ko = g[64 * hh:64 * (hh + 1), 0:SFREE]
                        for kk in range(2):
                            nc.tensor.matmul(
                                ko, w2k[:, kk, :],
                                ht[:, 1024 * kk + 512 * hh:
                                   1024 * kk + 512 * (hh + 1)],
                                start=(kk == 0), stop=(kk == 1))
                    kp = g[:, 0:SFREE]
                    if i < 3:
                        kb = kbpool.tile([128, SFREE], fp16, tag="kb", name="kb")
                        nc.vector.tensor_copy(kb, kp)
                        kb_prev[s] = kb
                    if i == 0:
                        a = accpool.tile([128, SFREE], f32, tag="acc", name="acc")
                        nc.vector.scalar_tensor_tensor(
                            a, kp, ci, ys[s], op0=ALU.mult, op1=ALU.add)
                        acc[s] = a
                    elif i < 3:
                        nc.vector.scalar_tensor_tensor(
                            acc[s], kp, ci, acc[s], op0=ALU.mult, op1=ALU.add)
                    else:
                        nc.vector.scalar_tensor_tensor(
                            ys[s], kp, ci, acc[s], op0=ALU.mult, op1=ALU.add)
                        nc.vector.tensor_copy(yr[s], ys[s])

        project_and_store(out_d[0:1])
        out_shift = out_d[1:T]
        if n_intervals > 1:
            from concourse.bass import ds
            with tc.For_i(0, n_intervals, 1,
                          hint_engines=(ET.PE, ET.Activation, ET.DVE)) as iv:
                for _ in range(N_SUB):
                    substep(h6col, h3col)
                project_and_store(out_shift[ds(iv, 1)])
        else:
            for _ in range(N_SUB):
                substep(h6col, h3col)
            project_and_store(out_shift[0:1])

    _split_multiwait_instructions(nc)
    return nc


def _prep_inputs(y0, t, W1, b1, W2, b2, Wfc, bfc):
    t = np.asarray(t, np.float32)
    dts = t[1:].astype(np.float64) - t[:-1].astype(np.float64)
    assert np.allclose(dts, dts[0]), "kernel assumes uniform time grid"
    h = float(np.float32(t[1] - t[0]) / np.float32(N_SUB))

    W1 = np.asarray(W1, np.float32)
    W2 = np.asarray(W2, np.float32)
    b1 = np.asarray(b1, np.float32)
    b2 = np.asarray(b2, np.float32)
    Wfc = np.asarray(Wfc, np.float32)
    bfc = np.asarray(bfc, np.float32)
    assert not np.any(bfc), "nonzero bfc not wired (always zero here)"
    assert not np.any(b1), "nonzero b1 not wired (always zero here)"
    assert not np.any(b2), "nonzero b2 not wired (always zero here)"

    def stackp(a):  # [64, X] -> [128, X]
        return np.ascontiguousarray(np.concatenate([a, a], axis=0))

    w1h2 = stackp(W1 * np.float32(h / 2)).astype(np.float16)
    w1h1 = stackp(W1 * np.float32(h)).astype(np.float16)

    def w2pack(a):  # [256, 64] -> [128, 2, 64]
        return np.ascontiguousarray(a.reshape(2, 128, D).transpose(1, 0, 2))

    w2k = w2pack(W2).astype(np.float16)

    wfcs = stackp(Wfc).astype(np.float32)
    ones = np.ones((128, 1), np.float32)
    h6col = ones * np.float32(h / 6)
    h3col = ones * np.float32(h / 3)

    y0 = np.asarray(y0, np.float32)
    rblob = stackp(W1).astype(np.float32)
    bblob = np.concatenate([w1h2, w1h1, w2k.reshape(128, 2 * D)], axis=1)
    in_maps = []
    for c in range(N_CORES):
        shard = y0[c * B_CORE:(c + 1) * B_CORE]               # [2048, 64]
        yT = np.ascontiguousarray(shard.T)                    # [64, 2048]
        y0p = np.concatenate([yT[:, :HALF], yT[:, HALF:]], axis=0)
        fblob = np.concatenate([wfcs, h6col, h3col,
                                np.ascontiguousarray(y0p)], axis=1)
        in_maps.append({"rblob": np.ascontiguousarray(rblob),
                        "fblob": np.ascontiguousarray(fblob),
                        "bblob": np.ascontiguousarray(bblob)})
    return in_maps, h


_KERNEL_CACHE = {}


def _get_kernel(n_intervals, **kw):
    key = (n_intervals, tuple(sorted(kw.items())))
    if key not in _KERNEL_CACHE:
        _KERNEL_CACHE[key] = _build_kernel(n_intervals, **kw)
    return _KERNEL_CACHE[key]


def _run(inputs, n_intervals=T_FULL - 1, trace=False, **kw):
    from concourse import bass_utils

    in_maps, _ = _prep_inputs(**inputs)
    nc = _get_kernel(n_intervals)
    return bass_utils.run_bass_kernel_spmd(
        nc, in_maps, list(range(N_CORES)), trace=trace, **kw)


def _unstage(o):
    # [T, 128, 512] staged -> [T, B_CORE, O]
    # partition p = 32*(2s+hh) + o, col c -> batch b = 1024*hh + 512*s + c
    T = o.shape[0]
    a = o.reshape(T, 2, 2, O, SFREE)            # [t, s, hh, o, c]
    return np.ascontiguousarray(
        a.transpose(0, 2, 1, 4, 3).reshape(T, B_CORE, O))


def kernel(y0, t, W1, b1, W2, b2, Wfc, bfc):
    res = _run(dict(y0=y0, t=t, W1=W1, b1=b1, W2=W2, b2=b2, Wfc=Wfc, bfc=bfc))
    full = np.concatenate(
        [_unstage(res.results[c]["out"]) for c in range(N_CORES)], axis=1)
    return np.ascontiguousarray(full.astype(np.float32))
